# revision 6
# baseline (speedup 1.0000x reference)
"""Trainium2 Bass kernel for CapsNet conv + dynamic-routing block.

Math note: in the reference, `pred` has a singleton MI axis, so the
softmax-weighted sum over MI is `pred` itself for any routing logits
(softmax rows sum to 1), and the `b` updates never change `c`.  The whole
module therefore reduces exactly to

    out = squash(conv2d_3x3(x2, conv_w) + conv_b)   # squash over DO

with x2 = x reshaped [B, MI*DI, H, W] and output [B, MO, H, W, DO].

Strategy: data-parallel over batch (1 image per NeuronCore, 8 cores).
Per core the conv runs as 9 accumulating bf16 matmuls per 512-pixel chunk
([ci,co] stationary, shifted window of a host-prepadded bf16 image moving),
keeping the PE stream pure matmul.  Everything else is off the PE and
batched per chunk-pair to amortize instruction/DMA-issue overheads:
  - bias add + f32->bf16 cast on ACT (PSUM -> SBUF),
  - [co,pix] -> [pix,co] transpose via the DMA crossbar (bf16, 1/pair),
  - squash: square on Pool, grouped reduce + factor on DVE (+ACT sqrt),
    final multiply alternating DVE/Pool, bf16 output (host upcasts),
  - DMA split across the SP and ACT HWDGE rings.
"""

from contextlib import ExitStack

import numpy as np

import concourse.bass as bass
import concourse.mybir as mybir
import concourse.tile as tile
from concourse import bacc
from concourse.bass_utils import run_bass_kernel_spmd

B, MI, H, W, DI = 8, 8, 64, 64, 16
MO, DO = 8, 16
CI = MI * DI  # 128
CO = MO * DO  # 128
P = 128
HP, WP = H + 2, W + 2  # 66 (zero pad = 1, baked in on host)
NPAIR = 4  # 1024-pixel chunk-pairs per 64x64 image
EPS = 1e-7

F32 = mybir.dt.float32
BF16 = mybir.dt.bfloat16


def _body(tc, x_in, w_in, b_in, out_d, reps=1):
    import os

    variant = os.environ.get("KVAR", "full")
    nc = tc.nc
    with ExitStack() as ctx:
        consts = ctx.enter_context(tc.tile_pool(name="consts", bufs=1))
        cpsum = ctx.enter_context(tc.tile_pool(name="cpsum", bufs=6, space="PSUM"))
        spool = ctx.enter_context(tc.tile_pool(name="spool", bufs=3))
        sopool = ctx.enter_context(tc.tile_pool(name="sopool", bufs=3))
        sqpool = ctx.enter_context(tc.tile_pool(name="sqpool", bufs=3))
        facpool = ctx.enter_context(tc.tile_pool(name="facpool", bufs=3))
        outp = ctx.enter_context(tc.tile_pool(name="outp", bufs=3))

        # weights: [ci, s, co] bf16 in SBUF (ACT ring, parallel with x on SP)
        w_sb = consts.tile([P, 9, CO], BF16)
        nc.scalar.dma_start(w_sb[:], w_in.rearrange("s ci co -> ci s co"))

        bias_sb = consts.tile([P, 1], F32)
        nc.scalar.dma_start(bias_sb[:], b_in)

        eps_sb = consts.tile([P, 1], F32)
        nc.vector.memset(eps_sb[:], EPS)

        # two host-prepadded images (bf16) so DMA-in of the next image
        # overlaps the current image's conv.
        xpads = [
            consts.tile([P, HP, WP], BF16, tag=n, name=n) for n in ("xpa", "xpb")
        ]

        def load_half(xp, g):
            # rows 33g .. 33g+33, fully contiguous in DRAM and SBUF
            nc.scalar.dma_start(
                xp[:, 33 * g : 33 * g + 33, :],
                x_in[:, 33 * WP * g : 33 * WP * g + 33 * WP].rearrange(
                    "ci (r w) -> ci r w", w=WP
                ),
            )

        def conv_pair(xp, pr):
            # conv for chunks 2pr, 2pr+1: 9 accumulating bf16 matmuls each,
            # s-outer so each weight load is reused back-to-back.
            c0 = 2 * pr
            ps0 = cpsum.tile([P, 4 * P], F32, tag="ps")
            ps1 = cpsum.tile([P, 4 * P], F32, tag="ps")
            nmm = 9 if variant != "dmaonly" else 1
            for s in range(nmm):
                kh, kw = s // 3, s % 3
                for ps, c in ((ps0, c0), (ps1, c0 + 1)):
                    rhs = xp[:, 8 * c + kh : 8 * c + kh + 8, kw : kw + 64]
                    nc.tensor.matmul(
                        ps[:],
                        w_sb[:, s, :],
                        rhs,
                        start=(s == 0),
                        stop=(s == nmm - 1),
                    )
            return ps0, ps1

        def post_a(pr, out_slot, ps0, ps1):
            """Per-pair front half: bias add, transpose, square, reduce."""
            # bias add + cast to bf16 on ACT (PSUM -> SBUF), both chunks into
            # one contiguous pair tile
            s_pair = spool.tile([P, 2, 4 * P], BF16, tag="s_pair")
            nc.scalar.add(s_pair[:, 0], ps0[:], bias_sb[:])
            nc.scalar.add(s_pair[:, 1], ps1[:], bias_sb[:])

            if variant in ("convonly", "dmaonly"):
                o = outp.tile([P, 8, P], BF16, tag="out")
                nc.vector.tensor_copy(
                    o[:], s_pair[:].rearrange("p two (j co) -> p (two j) co", co=CO)
                )
                if variant != "nodma":
                    nc.sync.dma_start(out_slot, o[:])
                return None

            # transpose [co, 1024pix] -> [pix, 8, co] on the DMA crossbar
            so = sopool.tile([P, 8, P], BF16, tag="so")
            nc.sync.dma_start_transpose(
                so[:], s_pair[:].rearrange("p two q -> p (two q)")
            )

            # square on ACT (bf16), grouped sum over DO on DVE (f32)
            sq = sqpool.tile([P, 8, P], BF16, tag="sq")
            nc.scalar.square(sq[:], so[:])
            red = facpool.tile([P, 8 * MO], F32, tag="red")
            nc.vector.tensor_reduce(
                red[:],
                sq[:].rearrange("p j (mo do) -> p (j mo) do", do=DO),
                axis=mybir.AxisListType.X,
                op=mybir.AluOpType.add,
            )
            return (out_slot, so, red)

        def post_b(ctx):
            """Per-pair back half: factor + final multiply + store, issued one
            pair late so no engine queue blocks on the factor chain."""
            if ctx is None:
                return
            out_slot, so, red = ctx
            # factor = red / ((1+red) * sqrt(red+eps))
            r = facpool.tile([P, 8 * MO], F32, tag="r")
            nc.scalar.activation(
                r[:], red[:], mybir.ActivationFunctionType.Sqrt, bias=eps_sb[:]
            )
            d = facpool.tile([P, 8 * MO], F32, tag="d")
            nc.vector.scalar_tensor_tensor(
                d[:], red[:], 1.0, r[:], mybir.AluOpType.add, mybir.AluOpType.mult
            )
            rcp = facpool.tile([P, 8 * MO], F32, tag="rcp")
            nc.vector.reciprocal(rcp[:], d[:])
            fac = facpool.tile([P, 8 * MO], F32, tag="fac")
            nc.vector.tensor_mul(fac[:], red[:], rcp[:])

            # final multiply on Pool (its queue holds only these)
            o = outp.tile([P, 8, P], BF16, tag="out")
            nc.gpsimd.tensor_mul(
                o[:].rearrange("p j (mo do) -> p (j mo) do", do=DO),
                so[:].rearrange("p j (mo do) -> p (j mo) do", do=DO),
                fac[:, :, None].to_broadcast((P, 8 * MO, DO)),
            )
            if variant != "nodma":
                nc.sync.dma_start(out_slot, o[:])

        def one_image(xp, prefetch, pending):
            """prefetch: list of 4 callables (or None), one per pair.
            pending: post_b context carried from the previous pair."""
            for pr in range(NPAIR):
                if prefetch[pr] is not None and variant != "nodma":
                    prefetch[pr]()
                ps0, ps1 = conv_pair(xp, pr)
                ctx = post_a(pr, out_d[:, pr], ps0, ps1)
                post_b(pending)
                pending = ctx
            return pending

        xa, xb = xpads
        NOPREF = [None, None, None, None]

        if reps == 1:
            if variant != "nodma":
                load_half(xa, 0)
                load_half(xa, 1)
            post_b(one_image(xa, NOPREF, None))
        else:
            # UNROLL images per hardware-loop iteration (amortizes the For_i
            # all-engine barrier); next image's halves prefetch during conv.
            UNROLL = 8

            def body():
                pending = None
                for k in range(UNROLL):
                    xp = (xa, xb)[k % 2]
                    xn = (xa, xb)[(k + 1) % 2]
                    pref = [
                        (lambda xn=xn: load_half(xn, 0)),
                        (lambda xn=xn: load_half(xn, 1)),
                        None,
                        None,
                    ]
                    pending = one_image(xp, pref, pending)
                post_b(pending)

            if variant != "nodma":
                load_half(xa, 0)
                load_half(xa, 1)
            if reps < 0:
                for _ in range((-reps) // UNROLL):
                    body()
            else:
                with tc.For_i(0, reps // UNROLL, 1):
                    body()


_NC_CACHE = {}


def _get_nc(reps=1):
    key = ("nc", reps)
    if key not in _NC_CACHE:
        nc = bacc.Bacc("TRN2", target_bir_lowering=False, debug=False, num_devices=8)
        x_in = nc.dram_tensor("x", [CI, HP * WP], BF16, kind="ExternalInput").ap()
        w_in = nc.dram_tensor("w", [9, CI, CO], BF16, kind="ExternalInput").ap()
        b_in = nc.dram_tensor("bias", [CO, 1], F32, kind="ExternalInput").ap()
        out_d = nc.dram_tensor(
            "out", [P, NPAIR, 8, CO], BF16, kind="ExternalOutput"
        ).ap()
        with tile.TileContext(nc) as tc:
            _body(tc, x_in, w_in, b_in, out_d, reps=reps)
        nc.compile()
        _NC_CACHE[key] = nc
    return _NC_CACHE[key]


def run(x, conv_w, conv_b, trace=False, reps=1):
    import ml_dtypes

    nc = _get_nc(reps=reps)
    # shard/prep: channel-major x per image, zero-padded on host, bf16
    xt = (
        np.asarray(x, dtype=np.float32)
        .transpose(0, 1, 4, 2, 3)
        .reshape(B, CI, H, W)
    )
    xp = np.zeros((B, CI, HP, WP), dtype=ml_dtypes.bfloat16)
    xp[:, :, 1 : H + 1, 1 : W + 1] = xt.astype(ml_dtypes.bfloat16)
    xp = np.ascontiguousarray(xp.reshape(B, CI, HP * WP))
    w9 = np.ascontiguousarray(
        np.asarray(conv_w, dtype=np.float32)
        .reshape(CO, CI, 9)
        .transpose(2, 1, 0)
        .astype(ml_dtypes.bfloat16)
    )
    bias = np.ascontiguousarray(np.asarray(conv_b, dtype=np.float32).reshape(CO, 1))
    in_maps = [{"x": xp[b], "w": w9, "bias": bias} for b in range(B)]
    res = run_bass_kernel_spmd(nc, in_maps, list(range(B)), trace=trace)
    # gather/unshard: out_dev[p, pr, j, mo, do] -> out[b, mo, h, w, do]
    # with pixel = 1024 pr + 128 j + p, h = 16 pr + 2 j + p//64, w = p%64
    dev = np.stack(
        [res.results[i]["out"].astype(np.float32) for i in range(B)], axis=0
    )
    dev = dev.reshape(B, 2, W, NPAIR, 8, MO, DO)  # [b, ph, w, pr, j, mo, do]
    out = np.ascontiguousarray(
        dev.transpose(0, 5, 3, 4, 1, 2, 6).reshape(B, MO, H, W, DO)
    )
    return out, res


def kernel(x, conv_w, conv_b, b_logits=None, **_ignored):
    # b_logits provably has no effect on the reference output (see module
    # docstring), so it is accepted and ignored.
    out, _ = run(x, conv_w, conv_b, trace=False)
    return out


# revision 19
# speedup vs baseline: 1.6368x; 1.6368x over previous
"""Trainium2 Bass kernel for CapsNet conv + dynamic-routing block.

Math note: in the reference, `pred` has a singleton MI axis, so the
softmax-weighted sum over MI is `pred` itself for any routing logits
(softmax rows sum to 1), and the `b` updates never change `c`.  The whole
module therefore reduces exactly to

    out = squash(conv2d_3x3(x2, conv_w) + conv_b)   # squash over DO

with x2 = x reshaped [B, MI*DI, H, W] and output [B, MO, H, W, DO].

Strategy: data-parallel over batch (1 image per NeuronCore, 8 cores).
Per core the conv runs as 9 accumulating bf16 matmuls per 512-pixel chunk
([ci,co] stationary, shifted window of a host-prepadded bf16 image moving),
keeping the PE stream pure matmul; bias-add PSUM->SBUF per chunk on ACT.
The whole squash tail is IMAGE-granular — one big op per engine per image
so no engine queue can head-block another:
  - one [co,4096] -> [pix,32,co] DMA-crossbar transpose (bf16),
  - one DVE square (all-bf16 2x mode), one DVE grouped reduce,
  - factor chain on [128,256] (ACT sqrt + 3 small DVE ops),
  - one Pool (gpsimd) final multiply, bf16 output (host upcasts),
  - input loads + output stores on the SP ring, transpose on the ACT ring.
"""

from contextlib import ExitStack

import numpy as np

import concourse.bass as bass
import concourse.mybir as mybir
import concourse.tile as tile
from concourse import bacc
from concourse.bass_utils import run_bass_kernel_spmd

B, MI, H, W, DI = 8, 8, 64, 64, 16
MO, DO = 8, 16
CI = MI * DI  # 128
CO = MO * DO  # 128
P = 128
HP, WP = H + 2, W + 2  # 66 (zero pad = 1, baked in on host)
NPAIR = 4  # 1024-pixel chunk-pairs per 64x64 image
EPS = 1e-7

F32 = mybir.dt.float32
BF16 = mybir.dt.bfloat16


def _body(tc, x_in, w_in, b_in, out_d, reps=1):
    import os

    variant = os.environ.get("KVAR", "full")
    nc = tc.nc
    with ExitStack() as ctx:
        consts = ctx.enter_context(tc.tile_pool(name="consts", bufs=1))
        cpsum = ctx.enter_context(tc.tile_pool(name="cpsum", bufs=8, space="PSUM"))
        spool = ctx.enter_context(tc.tile_pool(name="spool", bufs=2))
        sopool = ctx.enter_context(tc.tile_pool(name="sopool", bufs=3))
        sqpool = ctx.enter_context(tc.tile_pool(name="sqpool", bufs=2))
        facpool = ctx.enter_context(tc.tile_pool(name="facpool", bufs=3))
        outp = ctx.enter_context(tc.tile_pool(name="outp", bufs=2))

        # weights: [ci, s, co] bf16 in SBUF (ACT ring, parallel with x on SP)
        w_sb = consts.tile([P, 9, CO], BF16)
        nc.scalar.dma_start(w_sb[:], w_in.rearrange("s ci co -> ci s co"))

        bias_sb = consts.tile([P, 1], F32)
        nc.scalar.dma_start(bias_sb[:], b_in)

        eps_sb = consts.tile([P, 1], F32)
        nc.vector.memset(eps_sb[:], EPS)

        # two host-prepadded images (bf16) so DMA-in of the next image
        # overlaps the current image's conv.
        xpads = [
            consts.tile([P, HP, WP], BF16, tag=n, name=n) for n in ("xpa", "xpb")
        ]

        def load_half(xp, g):
            # rows 33g .. 33g+33, fully contiguous in DRAM and SBUF
            nc.sync.dma_start(
                xp[:, 33 * g : 33 * g + 33, :],
                x_in[:, 33 * WP * g : 33 * WP * g + 33 * WP].rearrange(
                    "ci (r w) -> ci r w", w=WP
                ),
            )

        def conv_pair(xp, s_img, pr):
            # conv for chunks 2pr, 2pr+1: 9 accumulating bf16 matmuls each,
            # s-outer so each weight load is reused back-to-back; bias-add
            # drains each chunk's PSUM bank into the image tile on ACT.
            c0 = 2 * pr
            ps0 = cpsum.tile([P, 4 * P], F32, tag="ps")
            ps1 = cpsum.tile([P, 4 * P], F32, tag="ps")
            nmm = 9 if variant != "dmaonly" else 1
            for s in range(nmm):
                kh, kw = s // 3, s % 3
                for ps, c in ((ps0, c0), (ps1, c0 + 1)):
                    rhs = xp[:, 8 * c + kh : 8 * c + kh + 8, kw : kw + 64]
                    nc.tensor.matmul(
                        ps[:],
                        w_sb[:, s, :],
                        rhs,
                        start=(s == 0),
                        stop=(s == nmm - 1),
                    )
            nc.scalar.add(s_img[:, c0], ps0[:], bias_sb[:])
            nc.scalar.add(s_img[:, c0 + 1], ps1[:], bias_sb[:])

        def one_image(xp, prefetch):
            """prefetch: list of 4 callables (or None), one per pair."""
            s_img = spool.tile([P, 8, 4 * P], BF16, tag="s_img")
            for pr in range(NPAIR):
                if prefetch[pr] is not None and variant != "nodma":
                    prefetch[pr]()
                conv_pair(xp, s_img, pr)

            if variant in ("convonly", "dmaonly"):
                if variant != "nodma":
                    nc.sync.dma_start(
                        out_d[:],
                        s_img[:].rearrange("p c (j co) -> p (c j) co", co=CO)
                        .rearrange("p (pr j) co -> p pr j co", j=8),
                    )
                return

            # one whole-image transpose [co, 4096] -> [pix, 32, co] (ACT ring)
            so = sopool.tile([P, 32, P], BF16, tag="so")
            nc.scalar.dma_start_transpose(
                so[:], s_img[:].rearrange("p c q -> p (c q)")
            )

            # square (all-bf16 DVE 2x mode), grouped sum over DO (f32)
            sq = sqpool.tile([P, 32, P], BF16, tag="sq")
            nc.vector.tensor_mul(sq[:], so[:], so[:])
            red = facpool.tile([P, 32 * MO], F32, tag="red")
            nc.vector.tensor_reduce(
                red[:],
                sq[:].rearrange("p j (mo do) -> p (j mo) do", do=DO),
                axis=mybir.AxisListType.X,
                op=mybir.AluOpType.add,
            )

            # factor = red / ((1+red) * sqrt(red+eps))
            r = facpool.tile([P, 32 * MO], F32, tag="r")
            nc.scalar.activation(
                r[:], red[:], mybir.ActivationFunctionType.Sqrt, bias=eps_sb[:]
            )
            d = facpool.tile([P, 32 * MO], F32, tag="d")
            nc.vector.scalar_tensor_tensor(
                d[:], red[:], 1.0, r[:], mybir.AluOpType.add, mybir.AluOpType.mult
            )
            rcp = facpool.tile([P, 32 * MO], F32, tag="rcp")
            nc.vector.reciprocal(rcp[:], d[:])
            fac = facpool.tile([P, 32 * MO], F32, tag="fac")
            nc.vector.tensor_mul(fac[:], red[:], rcp[:])

            # final multiply on Pool (gpsimd) — its queue holds only these
            o = outp.tile([P, 32, P], BF16, tag="out")
            nc.gpsimd.tensor_mul(
                o[:].rearrange("p j (mo do) -> p (j mo) do", do=DO),
                so[:].rearrange("p j (mo do) -> p (j mo) do", do=DO),
                fac[:, :, None].to_broadcast((P, 32 * MO, DO)),
            )
            if variant != "nodma":
                nc.sync.dma_start(
                    out_d[:], o[:].rearrange("p (pr j) co -> p pr j co", j=8)
                )

        xa, xb = xpads
        NOPREF = [None, None, None, None]

        if reps == 1:
            if variant != "nodma":
                load_half(xa, 0)
                load_half(xa, 1)
            one_image(xa, NOPREF)
        else:
            # UNROLL images per hardware-loop iteration (amortizes the For_i
            # all-engine barrier); next image's halves prefetch during conv.
            UNROLL = 16

            def body():
                for k in range(UNROLL):
                    xp = (xa, xb)[k % 2]
                    xn = (xa, xb)[(k + 1) % 2]
                    pref = [
                        (lambda xn=xn: load_half(xn, 0)),
                        (lambda xn=xn: load_half(xn, 1)),
                        None,
                        None,
                    ]
                    one_image(xp, pref)

            if variant != "nodma":
                load_half(xa, 0)
                load_half(xa, 1)
            if reps < 0:
                for _ in range((-reps) // UNROLL):
                    body()
            else:
                with tc.For_i(0, reps // UNROLL, 1):
                    body()


_NC_CACHE = {}


def _get_nc(reps=1):
    key = ("nc", reps)
    if key not in _NC_CACHE:
        nc = bacc.Bacc("TRN2", target_bir_lowering=False, debug=False, num_devices=8)
        x_in = nc.dram_tensor("x", [CI, HP * WP], BF16, kind="ExternalInput").ap()
        w_in = nc.dram_tensor("w", [9, CI, CO], BF16, kind="ExternalInput").ap()
        b_in = nc.dram_tensor("bias", [CO, 1], F32, kind="ExternalInput").ap()
        out_d = nc.dram_tensor(
            "out", [P, NPAIR, 8, CO], BF16, kind="ExternalOutput"
        ).ap()
        with tile.TileContext(nc) as tc:
            _body(tc, x_in, w_in, b_in, out_d, reps=reps)
        nc.compile()
        _NC_CACHE[key] = nc
    return _NC_CACHE[key]


def run(x, conv_w, conv_b, trace=False, reps=1):
    import ml_dtypes

    nc = _get_nc(reps=reps)
    # shard/prep: channel-major x per image, zero-padded on host, bf16
    xt = (
        np.asarray(x, dtype=np.float32)
        .transpose(0, 1, 4, 2, 3)
        .reshape(B, CI, H, W)
    )
    xp = np.zeros((B, CI, HP, WP), dtype=ml_dtypes.bfloat16)
    xp[:, :, 1 : H + 1, 1 : W + 1] = xt.astype(ml_dtypes.bfloat16)
    xp = np.ascontiguousarray(xp.reshape(B, CI, HP * WP))
    w9 = np.ascontiguousarray(
        np.asarray(conv_w, dtype=np.float32)
        .reshape(CO, CI, 9)
        .transpose(2, 1, 0)
        .astype(ml_dtypes.bfloat16)
    )
    bias = np.ascontiguousarray(np.asarray(conv_b, dtype=np.float32).reshape(CO, 1))
    in_maps = [{"x": xp[b], "w": w9, "bias": bias} for b in range(B)]
    res = run_bass_kernel_spmd(nc, in_maps, list(range(B)), trace=trace)
    # gather/unshard: out_dev[p, pr, j, mo, do] -> out[b, mo, h, w, do]
    # with pixel = 1024 pr + 128 j + p, h = 16 pr + 2 j + p//64, w = p%64
    dev = np.stack(
        [res.results[i]["out"].astype(np.float32) for i in range(B)], axis=0
    )
    dev = dev.reshape(B, 2, W, NPAIR, 8, MO, DO)  # [b, ph, w, pr, j, mo, do]
    out = np.ascontiguousarray(
        dev.transpose(0, 5, 3, 4, 1, 2, 6).reshape(B, MO, H, W, DO)
    )
    return out, res


def kernel(x, conv_w, conv_b, b_logits=None, **_ignored):
    # b_logits provably has no effect on the reference output (see module
    # docstring), so it is accepted and ignored.
    out, _ = run(x, conv_w, conv_b, trace=False)
    return out


# revision 23
# speedup vs baseline: 1.9122x; 1.1683x over previous
"""Trainium2 Bass kernel for CapsNet conv + dynamic-routing block.

Math note: in the reference, `pred` has a singleton MI axis, so the
softmax-weighted sum over MI is `pred` itself for any routing logits
(softmax rows sum to 1), and the `b` updates never change `c`.  The whole
module therefore reduces exactly to

    out = squash(conv2d_3x3(x2, conv_w) + conv_b)   # squash over DO

with x2 = x reshaped [B, MI*DI, H, W] and output [B, MO, H, W, DO].

Strategy: data-parallel over batch (1 image per NeuronCore, 8 cores).
Per core the conv runs as 9 accumulating bf16 matmuls per 512-pixel chunk
([ci,co] stationary, shifted window of a host-prepadded bf16 image moving),
keeping the PE stream pure matmul; bias-add PSUM->SBUF per chunk on ACT.
The whole squash tail is IMAGE-granular — one big op per engine per image
so no engine queue can head-block another:
  - one [co,4096] -> [pix,32,co] DMA-crossbar transpose (bf16),
  - one DVE square (all-bf16 2x mode), one DVE grouped reduce,
  - factor chain on [128,256] (ACT sqrt + 3 small DVE ops),
  - one Pool (gpsimd) final multiply, bf16 output (host upcasts),
  - input loads + output stores on the SP ring, transpose on the ACT ring.
"""

from contextlib import ExitStack

import numpy as np

import concourse.bass as bass
import concourse.mybir as mybir
import concourse.tile as tile
from concourse import bacc
from concourse.bass_utils import run_bass_kernel_spmd

B, MI, H, W, DI = 8, 8, 64, 64, 16
MO, DO = 8, 16
CI = MI * DI  # 128
CO = MO * DO  # 128
P = 128
HP, WP = H + 2, W + 2  # 66 (zero pad = 1, baked in on host)
NPAIR = 4  # 1024-pixel chunk-pairs per 64x64 image
EPS = 1e-7

F32 = mybir.dt.float32
BF16 = mybir.dt.bfloat16


def _body(tc, x_in, w_in, b_in, out_d, reps=1):
    import os

    variant = os.environ.get("KVAR", "full")
    nc = tc.nc
    with ExitStack() as ctx:
        consts = ctx.enter_context(tc.tile_pool(name="consts", bufs=1))
        cpsum = ctx.enter_context(tc.tile_pool(name="cpsum", bufs=8, space="PSUM"))
        spool = ctx.enter_context(tc.tile_pool(name="spool", bufs=3))
        sopool = ctx.enter_context(tc.tile_pool(name="sopool", bufs=4))
        sqpool = ctx.enter_context(tc.tile_pool(name="sqpool", bufs=3))
        facpool = ctx.enter_context(tc.tile_pool(name="facpool", bufs=3))
        outp = ctx.enter_context(tc.tile_pool(name="outp", bufs=3))

        # weights: [ci, s, co] bf16 in SBUF (ACT ring, parallel with x on SP)
        w_sb = consts.tile([P, 9, CO], BF16)
        nc.scalar.dma_start(w_sb[:], w_in.rearrange("s ci co -> ci s co"))

        bias_sb = consts.tile([P, 1], F32)
        nc.scalar.dma_start(bias_sb[:], b_in)

        eps_sb = consts.tile([P, 1], F32)
        nc.vector.memset(eps_sb[:], EPS)

        # two host-prepadded images (bf16) so DMA-in of the next image
        # overlaps the current image's conv.
        xpads = [
            consts.tile([P, HP, WP], BF16, tag=n, name=n) for n in ("xpa", "xpb")
        ]

        def load_half(xp, g):
            # rows 33g .. 33g+33, fully contiguous in DRAM and SBUF
            nc.sync.dma_start(
                xp[:, 33 * g : 33 * g + 33, :],
                x_in[:, 33 * WP * g : 33 * WP * g + 33 * WP].rearrange(
                    "ci (r w) -> ci r w", w=WP
                ),
            )

        def conv_pair(xp, s_img, pr):
            # conv for chunks 2pr, 2pr+1: 9 accumulating bf16 matmuls each,
            # s-outer so each weight load is reused back-to-back; bias-add
            # drains each chunk's PSUM bank into the image tile on ACT.
            c0 = 2 * pr
            ps0 = cpsum.tile([P, 4 * P], F32, tag="ps")
            ps1 = cpsum.tile([P, 4 * P], F32, tag="ps")
            nmm = 9 if variant != "dmaonly" else 1
            for s in range(nmm):
                kh, kw = s // 3, s % 3
                for ps, c in ((ps0, c0), (ps1, c0 + 1)):
                    rhs = xp[:, 8 * c + kh : 8 * c + kh + 8, kw : kw + 64]
                    nc.tensor.matmul(
                        ps[:],
                        w_sb[:, s, :],
                        rhs,
                        start=(s == 0),
                        stop=(s == nmm - 1),
                    )
            nc.scalar.add(s_img[:, c0], ps0[:], bias_sb[:])
            nc.scalar.add(s_img[:, c0 + 1], ps1[:], bias_sb[:])

        def one_image(xp, prefetch):
            """prefetch: list of 4 callables (or None), one per pair."""
            s_img = spool.tile([P, 8, 4 * P], BF16, tag="s_img")
            for pr in range(NPAIR):
                if prefetch[pr] is not None and variant != "nodma":
                    prefetch[pr]()
                conv_pair(xp, s_img, pr)

            if variant in ("convonly", "dmaonly"):
                if variant != "nodma":
                    nc.sync.dma_start(
                        out_d[:],
                        s_img[:].rearrange("p c (j co) -> p (c j) co", co=CO)
                        .rearrange("p (pr j) co -> p pr j co", j=8),
                    )
                return

            # one whole-image transpose [co, 4096] -> [pix, 32, co] (ACT ring)
            so = sopool.tile([P, 32, P], BF16, tag="so")
            nc.scalar.dma_start_transpose(
                so[:], s_img[:].rearrange("p c q -> p (c q)")
            )

            # square (all-bf16 DVE 2x mode), grouped sum over DO (f32)
            sq = sqpool.tile([P, 32, P], BF16, tag="sq")
            nc.vector.tensor_mul(sq[:], so[:], so[:])
            red = facpool.tile([P, 32 * MO], F32, tag="red")
            nc.vector.tensor_reduce(
                red[:],
                sq[:].rearrange("p j (mo do) -> p (j mo) do", do=DO),
                axis=mybir.AxisListType.X,
                op=mybir.AluOpType.add,
            )

            # factor = red / ((1+red) * sqrt(red+eps))
            r = facpool.tile([P, 32 * MO], F32, tag="r")
            nc.scalar.activation(
                r[:], red[:], mybir.ActivationFunctionType.Sqrt, bias=eps_sb[:]
            )
            d = facpool.tile([P, 32 * MO], F32, tag="d")
            nc.vector.scalar_tensor_tensor(
                d[:], red[:], 1.0, r[:], mybir.AluOpType.add, mybir.AluOpType.mult
            )
            rcp = facpool.tile([P, 32 * MO], F32, tag="rcp")
            nc.vector.reciprocal(rcp[:], d[:])
            fac = facpool.tile([P, 32 * MO], F32, tag="fac")
            nc.vector.tensor_mul(fac[:], red[:], rcp[:])

            # final multiply on Pool (gpsimd) — its queue holds only these
            o = outp.tile([P, 32, P], BF16, tag="out")
            nc.gpsimd.tensor_mul(
                o[:].rearrange("p j (mo do) -> p (j mo) do", do=DO),
                so[:].rearrange("p j (mo do) -> p (j mo) do", do=DO),
                fac[:, :, None].to_broadcast((P, 32 * MO, DO)),
            )
            if variant != "nodma":
                nc.sync.dma_start(
                    out_d[:], o[:].rearrange("p (pr j) co -> p pr j co", j=8)
                )

        xa, xb = xpads
        NOPREF = [None, None, None, None]

        if reps == 1:
            if variant != "nodma":
                load_half(xa, 0)
                load_half(xa, 1)
            one_image(xa, NOPREF)
        else:
            # UNROLL images per hardware-loop iteration (amortizes the For_i
            # all-engine barrier); next image's halves prefetch during conv.
            UNROLL = 32

            def body():
                for k in range(UNROLL):
                    xp = (xa, xb)[k % 2]
                    xn = (xa, xb)[(k + 1) % 2]
                    pref = [
                        (lambda xn=xn: load_half(xn, 0)),
                        (lambda xn=xn: load_half(xn, 1)),
                        None,
                        None,
                    ]
                    one_image(xp, pref)

            if variant != "nodma":
                load_half(xa, 0)
                load_half(xa, 1)
            if reps < 0:
                for _ in range((-reps) // UNROLL):
                    body()
            else:
                with tc.For_i(0, reps // UNROLL, 1):
                    body()


_NC_CACHE = {}


def _get_nc(reps=1):
    key = ("nc", reps)
    if key not in _NC_CACHE:
        nc = bacc.Bacc("TRN2", target_bir_lowering=False, debug=False, num_devices=8)
        x_in = nc.dram_tensor("x", [CI, HP * WP], BF16, kind="ExternalInput").ap()
        w_in = nc.dram_tensor("w", [9, CI, CO], BF16, kind="ExternalInput").ap()
        b_in = nc.dram_tensor("bias", [CO, 1], F32, kind="ExternalInput").ap()
        out_d = nc.dram_tensor(
            "out", [P, NPAIR, 8, CO], BF16, kind="ExternalOutput"
        ).ap()
        with tile.TileContext(nc) as tc:
            _body(tc, x_in, w_in, b_in, out_d, reps=reps)
        nc.compile()
        _NC_CACHE[key] = nc
    return _NC_CACHE[key]


def run(x, conv_w, conv_b, trace=False, reps=1):
    import ml_dtypes

    nc = _get_nc(reps=reps)
    # shard/prep: channel-major x per image, zero-padded on host, bf16
    xt = (
        np.asarray(x, dtype=np.float32)
        .transpose(0, 1, 4, 2, 3)
        .reshape(B, CI, H, W)
    )
    xp = np.zeros((B, CI, HP, WP), dtype=ml_dtypes.bfloat16)
    xp[:, :, 1 : H + 1, 1 : W + 1] = xt.astype(ml_dtypes.bfloat16)
    xp = np.ascontiguousarray(xp.reshape(B, CI, HP * WP))
    w9 = np.ascontiguousarray(
        np.asarray(conv_w, dtype=np.float32)
        .reshape(CO, CI, 9)
        .transpose(2, 1, 0)
        .astype(ml_dtypes.bfloat16)
    )
    bias = np.ascontiguousarray(np.asarray(conv_b, dtype=np.float32).reshape(CO, 1))
    in_maps = [{"x": xp[b], "w": w9, "bias": bias} for b in range(B)]
    res = run_bass_kernel_spmd(nc, in_maps, list(range(B)), trace=trace)
    # gather/unshard: out_dev[p, pr, j, mo, do] -> out[b, mo, h, w, do]
    # with pixel = 1024 pr + 128 j + p, h = 16 pr + 2 j + p//64, w = p%64
    dev = np.stack(
        [res.results[i]["out"].astype(np.float32) for i in range(B)], axis=0
    )
    dev = dev.reshape(B, 2, W, NPAIR, 8, MO, DO)  # [b, ph, w, pr, j, mo, do]
    out = np.ascontiguousarray(
        dev.transpose(0, 5, 3, 4, 1, 2, 6).reshape(B, MO, H, W, DO)
    )
    return out, res


def kernel(x, conv_w, conv_b, b_logits=None, **_ignored):
    # b_logits provably has no effect on the reference output (see module
    # docstring), so it is accepted and ignored.
    out, _ = run(x, conv_w, conv_b, trace=False)
    return out


# revision 24
# speedup vs baseline: 2.4683x; 1.2908x over previous
"""Trainium2 Bass kernel for CapsNet conv + dynamic-routing block.

Math note: in the reference, `pred` has a singleton MI axis, so the
softmax-weighted sum over MI is `pred` itself for any routing logits
(softmax rows sum to 1), and the `b` updates never change `c`.  The whole
module therefore reduces exactly to

    out = squash(conv2d_3x3(x2, conv_w) + conv_b)   # squash over DO

with x2 = x reshaped [B, MI*DI, H, W] and output [B, MO, H, W, DO].

Strategy: data-parallel over batch (1 image per NeuronCore, 8 cores).
Per core the conv runs as 9 accumulating bf16 matmuls per 512-pixel chunk
([ci,co] stationary, shifted window of a host-prepadded bf16 image moving),
keeping the PE stream pure matmul; bias-add PSUM->SBUF per chunk on ACT.
The whole squash tail is IMAGE-granular — one big op per engine per image
so no engine queue can head-block another:
  - one [co,4096] -> [pix,32,co] DMA-crossbar transpose (bf16),
  - one DVE square (all-bf16 2x mode), one DVE grouped reduce,
  - factor chain on [128,256] (ACT sqrt + 3 small DVE ops),
  - one Pool (gpsimd) final multiply, bf16 output (host upcasts),
  - input loads + output stores on the SP ring, transpose on the ACT ring.
"""

from contextlib import ExitStack

import numpy as np

import concourse.bass as bass
import concourse.mybir as mybir
import concourse.tile as tile
from concourse import bacc
from concourse.bass_utils import run_bass_kernel_spmd

B, MI, H, W, DI = 8, 8, 64, 64, 16
MO, DO = 8, 16
CI = MI * DI  # 128
CO = MO * DO  # 128
P = 128
HP, WP = H + 2, W + 2  # 66 (zero pad = 1, baked in on host)
NPAIR = 4  # 1024-pixel chunk-pairs per 64x64 image
EPS = 1e-7

F32 = mybir.dt.float32
BF16 = mybir.dt.bfloat16


def _body(tc, x_in, w_in, b_in, out_d, reps=1):
    import os

    variant = os.environ.get("KVAR", "full")
    nc = tc.nc
    with ExitStack() as ctx:
        consts = ctx.enter_context(tc.tile_pool(name="consts", bufs=1))
        cpsum = ctx.enter_context(tc.tile_pool(name="cpsum", bufs=8, space="PSUM"))
        spool = ctx.enter_context(tc.tile_pool(name="spool", bufs=3))
        sopool = ctx.enter_context(tc.tile_pool(name="sopool", bufs=4))
        sqpool = ctx.enter_context(tc.tile_pool(name="sqpool", bufs=3))
        facpool = ctx.enter_context(tc.tile_pool(name="facpool", bufs=3))
        outp = ctx.enter_context(tc.tile_pool(name="outp", bufs=3))

        # weights: [ci, s, co] bf16 in SBUF (ACT ring, parallel with x on SP)
        w_sb = consts.tile([P, 9, CO], BF16)
        nc.scalar.dma_start(w_sb[:], w_in.rearrange("s ci co -> ci s co"))

        bias_sb = consts.tile([P, 1], F32)
        nc.scalar.dma_start(bias_sb[:], b_in)

        eps_sb = consts.tile([P, 1], F32)
        nc.vector.memset(eps_sb[:], EPS)

        # two host-prepadded images (bf16) so DMA-in of the next image
        # overlaps the current image's conv.
        xpads = [
            consts.tile([P, HP, WP], BF16, tag=n, name=n) for n in ("xpa", "xpb")
        ]

        def load_half(xp, g):
            # rows 33g .. 33g+33, fully contiguous in DRAM and SBUF
            nc.sync.dma_start(
                xp[:, 33 * g : 33 * g + 33, :],
                x_in[:, 33 * WP * g : 33 * WP * g + 33 * WP].rearrange(
                    "ci (r w) -> ci r w", w=WP
                ),
            )

        def conv_pair(xp, s_img, pr):
            # conv for chunks 2pr, 2pr+1: 9 accumulating bf16 matmuls each,
            # s-outer so each weight load is reused back-to-back; bias-add
            # drains each chunk's PSUM bank into the image tile on ACT.
            c0 = 2 * pr
            ps0 = cpsum.tile([P, 4 * P], F32, tag="ps")
            ps1 = cpsum.tile([P, 4 * P], F32, tag="ps")
            nmm = 9 if variant != "dmaonly" else 1
            for s in range(nmm):
                kh, kw = s // 3, s % 3
                for ps, c in ((ps0, c0), (ps1, c0 + 1)):
                    rhs = xp[:, 8 * c + kh : 8 * c + kh + 8, kw : kw + 64]
                    nc.tensor.matmul(
                        ps[:],
                        w_sb[:, s, :],
                        rhs,
                        start=(s == 0),
                        stop=(s == nmm - 1),
                    )
            nc.scalar.add(s_img[:, c0], ps0[:], bias_sb[:])
            nc.scalar.add(s_img[:, c0 + 1], ps1[:], bias_sb[:])

        def one_image(xp, prefetch):
            """prefetch: list of 4 callables (or None), one per pair."""
            s_img = spool.tile([P, 8, 4 * P], BF16, tag="s_img")
            for pr in range(NPAIR):
                if prefetch[pr] is not None and variant != "nodma":
                    prefetch[pr]()
                conv_pair(xp, s_img, pr)

            if variant in ("convonly", "dmaonly"):
                if variant != "nodma":
                    nc.sync.dma_start(
                        out_d[:],
                        s_img[:].rearrange("p c (j co) -> p (c j) co", co=CO)
                        .rearrange("p (pr j) co -> p pr j co", j=8),
                    )
                return

            # one whole-image transpose [co, 4096] -> [pix, 32, co] (ACT ring)
            so = sopool.tile([P, 32, P], BF16, tag="so")
            nc.scalar.dma_start_transpose(
                so[:], s_img[:].rearrange("p c q -> p (c q)")
            )

            # square (all-bf16 DVE 2x mode), grouped sum over DO (f32)
            sq = sqpool.tile([P, 32, P], BF16, tag="sq")
            nc.vector.tensor_mul(sq[:], so[:], so[:])
            red = facpool.tile([P, 32 * MO], F32, tag="red")
            nc.vector.tensor_reduce(
                red[:],
                sq[:].rearrange("p j (mo do) -> p (j mo) do", do=DO),
                axis=mybir.AxisListType.X,
                op=mybir.AluOpType.add,
            )

            # factor = red / ((1+red) * sqrt(red+eps))
            r = facpool.tile([P, 32 * MO], F32, tag="r")
            nc.scalar.activation(
                r[:], red[:], mybir.ActivationFunctionType.Sqrt, bias=eps_sb[:]
            )
            d = facpool.tile([P, 32 * MO], F32, tag="d")
            nc.vector.scalar_tensor_tensor(
                d[:], red[:], 1.0, r[:], mybir.AluOpType.add, mybir.AluOpType.mult
            )
            rcp = facpool.tile([P, 32 * MO], F32, tag="rcp")
            nc.vector.reciprocal(rcp[:], d[:])
            fac = facpool.tile([P, 32 * MO], F32, tag="fac")
            nc.vector.tensor_mul(fac[:], red[:], rcp[:])

            # final multiply on Pool (gpsimd) — its queue holds only these
            o = outp.tile([P, 32, P], BF16, tag="out")
            muleng = nc.vector if variant == "dvemul" else nc.gpsimd
            muleng.tensor_mul(
                o[:].rearrange("p j (mo do) -> p (j mo) do", do=DO),
                so[:].rearrange("p j (mo do) -> p (j mo) do", do=DO),
                fac[:, :, None].to_broadcast((P, 32 * MO, DO)),
            )
            if variant != "nodma":
                nc.sync.dma_start(
                    out_d[:], o[:].rearrange("p (pr j) co -> p pr j co", j=8)
                )

        xa, xb = xpads
        NOPREF = [None, None, None, None]

        if reps == 1:
            if variant != "nodma":
                load_half(xa, 0)
                load_half(xa, 1)
            one_image(xa, NOPREF)
        else:
            # UNROLL images per hardware-loop iteration (amortizes the For_i
            # all-engine barrier); next image's halves prefetch during conv.
            UNROLL = 32

            def body():
                for k in range(UNROLL):
                    xp = (xa, xb)[k % 2]
                    xn = (xa, xb)[(k + 1) % 2]
                    pref = [
                        (lambda xn=xn: load_half(xn, 0)),
                        (lambda xn=xn: load_half(xn, 1)),
                        None,
                        None,
                    ]
                    one_image(xp, pref)

            if variant != "nodma":
                load_half(xa, 0)
                load_half(xa, 1)
            if reps < 0:
                for _ in range((-reps) // UNROLL):
                    body()
            else:
                with tc.For_i(0, reps // UNROLL, 1):
                    body()


_NC_CACHE = {}


def _get_nc(reps=1):
    key = ("nc", reps)
    if key not in _NC_CACHE:
        nc = bacc.Bacc("TRN2", target_bir_lowering=False, debug=False, num_devices=8)
        x_in = nc.dram_tensor("x", [CI, HP * WP], BF16, kind="ExternalInput").ap()
        w_in = nc.dram_tensor("w", [9, CI, CO], BF16, kind="ExternalInput").ap()
        b_in = nc.dram_tensor("bias", [CO, 1], F32, kind="ExternalInput").ap()
        out_d = nc.dram_tensor(
            "out", [P, NPAIR, 8, CO], BF16, kind="ExternalOutput"
        ).ap()
        with tile.TileContext(nc) as tc:
            _body(tc, x_in, w_in, b_in, out_d, reps=reps)
        nc.compile()
        _NC_CACHE[key] = nc
    return _NC_CACHE[key]


def run(x, conv_w, conv_b, trace=False, reps=1):
    import ml_dtypes

    nc = _get_nc(reps=reps)
    # shard/prep: channel-major x per image, zero-padded on host, bf16
    xt = (
        np.asarray(x, dtype=np.float32)
        .transpose(0, 1, 4, 2, 3)
        .reshape(B, CI, H, W)
    )
    xp = np.zeros((B, CI, HP, WP), dtype=ml_dtypes.bfloat16)
    xp[:, :, 1 : H + 1, 1 : W + 1] = xt.astype(ml_dtypes.bfloat16)
    xp = np.ascontiguousarray(xp.reshape(B, CI, HP * WP))
    w9 = np.ascontiguousarray(
        np.asarray(conv_w, dtype=np.float32)
        .reshape(CO, CI, 9)
        .transpose(2, 1, 0)
        .astype(ml_dtypes.bfloat16)
    )
    bias = np.ascontiguousarray(np.asarray(conv_b, dtype=np.float32).reshape(CO, 1))
    in_maps = [{"x": xp[b], "w": w9, "bias": bias} for b in range(B)]
    res = run_bass_kernel_spmd(nc, in_maps, list(range(B)), trace=trace)
    # gather/unshard: out_dev[p, pr, j, mo, do] -> out[b, mo, h, w, do]
    # with pixel = 1024 pr + 128 j + p, h = 16 pr + 2 j + p//64, w = p%64
    dev = np.stack(
        [res.results[i]["out"].astype(np.float32) for i in range(B)], axis=0
    )
    dev = dev.reshape(B, 2, W, NPAIR, 8, MO, DO)  # [b, ph, w, pr, j, mo, do]
    out = np.ascontiguousarray(
        dev.transpose(0, 5, 3, 4, 1, 2, 6).reshape(B, MO, H, W, DO)
    )
    return out, res


def kernel(x, conv_w, conv_b, b_logits=None, **_ignored):
    # b_logits provably has no effect on the reference output (see module
    # docstring), so it is accepted and ignored.
    out, _ = run(x, conv_w, conv_b, trace=False)
    return out
